# revision 1
# baseline (speedup 1.0000x reference)
"""Trainium2 Bass kernel for a 2-layer LSTM decoder (nn_Decoder).

Strategy: tensor-parallel over the hidden dimension across 8 NeuronCores.
Each core owns a 128-wide slice of H=1024 for both LSTM layers (its 512 of
the 4096 gate rows), and replicates the final fc layer so the autoregressive
input y needs no exchange.  The only cross-core traffic is an allgather of
each layer's hidden-state slice (128x64 fp32 = 32KB) per step, done with
direct SBUF->SBUF remote DMA (no HBM bounce, no ncfw collective floor).

Layouts are feature-on-partition / batch-on-free ("transposed") everywhere.
LSTM gate matmuls run moving-weight orientation: out[batch(64), gates(512)]
= lhsT(activations.T).T @ rhs(W.T) in bf16 (1 cycle/row at N=512), fp32 PSUM
accumulation; gates are then PE-transposed to [hid(128), batch(64)] so
sigmoid/tanh biases ride free on the scalar engine's per-partition bias
operand and the new h slice lands directly in allgather layout.  The fc
matmul runs weight-stationary and produces y.T in PSUM directly.  Cell
state c stays fp32 on its own core slice.

Each exchange is ONE remote_dma_broadcast to the 7 peers (XOR-relative
routing; empirically the D2D lanes (delta bit 2 set) deliver to delta^2 so
those rdests are pre-compensated).  The destination slot is sender-pid *
64 columns via a register access pattern, so every core runs an identical
program.  Descriptor generation is pre-staged one step ahead on the Q7;
only trigger_dma sits on the critical path.

Flow control: h buffers are double-buffered by step parity; the LSTM's own
data-dependency chain guarantees a sender cannot run 2 steps ahead of any
receiver, so no credit messages are needed.
"""

import sys

sys.path.insert(0, "/opt/trn_rl_repo")

import numpy as np

B = 64
H = 1024
OUT = 512
N_CORES = 8
HPC = H // N_CORES  # 128 hidden units per core
SOS_VALUE = -2.0

_CACHE = {}


def _build(seq, exchanges=True, outdma=True, rswaits=True, one_bcast=True):
    from concourse import bacc, bass, mybir

    dt = mybir.dt
    f32 = dt.float32
    bf16 = dt.bfloat16
    AF = mybir.ActivationFunctionType
    ALU = mybir.AluOpType

    nc = bacc.Bacc("TRN2", target_bir_lowering=False, debug=False,
                   num_devices=N_CORES)

    # ---- DRAM I/O (per-core shards prepared on host) ----
    w0_d = nc.dram_tensor("w0", [12 * 128, 512], bf16, kind="ExternalInput")
    w1_d = nc.dram_tensor("w1", [16 * 128, 512], bf16, kind="ExternalInput")
    wfc_d = nc.dram_tensor("wfc", [8 * 128, 512], bf16, kind="ExternalInput")
    b0_d = nc.dram_tensor("b0", [128, 4], f32, kind="ExternalInput")
    b1_d = nc.dram_tensor("b1", [128, 4], f32, kind="ExternalInput")
    bfc_d = nc.dram_tensor("bfc", [128, 4], f32, kind="ExternalInput")
    h0i_d = nc.dram_tensor("h0i", [1024, 64], bf16, kind="ExternalInput")
    h1i_d = nc.dram_tensor("h1i", [1024, 64], bf16, kind="ExternalInput")
    c0i_d = nc.dram_tensor("c0i", [128, 64], f32, kind="ExternalInput")
    c1i_d = nc.dram_tensor("c1i", [128, 64], f32, kind="ExternalInput")
    yi_d = nc.dram_tensor("yi", [512, 64], bf16, kind="ExternalInput")
    id_d = nc.dram_tensor("ident", [128, 128], f32, kind="ExternalInput")
    out_d = nc.dram_tensor("out", [seq, 128, 256], bf16, kind="ExternalOutput")

    # ---- SBUF ----
    w0 = nc.alloc_sbuf_tensor("w0s", [128, 12 * 512], bf16)
    w1 = nc.alloc_sbuf_tensor("w1s", [128, 16 * 512], bf16)
    wfc = nc.alloc_sbuf_tensor("wfcs", [128, 8 * 512], bf16)
    b0 = nc.alloc_sbuf_tensor("b0s", [128, 4], f32)
    b1 = nc.alloc_sbuf_tensor("b1s", [128, 4], f32)
    bfc = nc.alloc_sbuf_tensor("bfcs", [128, 4], f32)
    ident = nc.alloc_sbuf_tensor("idents", [128, 128], f32)
    h0T = [nc.alloc_sbuf_tensor(f"h0T{p}", [128, 512], bf16) for p in range(2)]
    h1T = [nc.alloc_sbuf_tensor(f"h1T{p}", [128, 512], bf16) for p in range(2)]
    xT = [nc.alloc_sbuf_tensor(f"xT{p}", [128, 256], bf16) for p in range(2)]
    cT = [nc.alloc_sbuf_tensor(f"cT{l}", [128, 64], f32) for l in range(2)]
    g0sb = nc.alloc_sbuf_tensor("g0sb", [64, 512], f32)
    g1sb = nc.alloc_sbuf_tensor("g1sb", [64, 512], f32)
    gl = [[nc.alloc_sbuf_tensor(f"g{l}_{n}", [128, 64], f32)
           for n in ("i", "f", "g", "o", "th", "t1", "t2")] for l in range(2)]

    # ---- PSUM ----
    ps_g0 = nc.alloc_psum_tensor("ps_g0", [64, 512], f32)
    ps_g1 = nc.alloc_psum_tensor("ps_g1", [64, 512], f32)
    ps_t0 = nc.alloc_psum_tensor("ps_t0", [128, 256], f32)
    ps_t1 = nc.alloc_psum_tensor("ps_t1", [128, 256], f32)
    ps_ty = nc.alloc_psum_tensor("ps_ty", [128, 256], f32)

    # ---- semaphores ----
    S = lambda n: nc.alloc_semaphore(n)
    init = S("init")
    pe_g0, pe_t0 = S("pe_g0"), S("pe_t0")
    pe_g1, pe_t1 = S("pe_g1"), S("pe_t1")
    pe_ty = S("pe_ty")
    act_g0, act_th0 = S("act_g0"), S("act_th0")
    act_g1, act_th1 = S("act_g1"), S("act_th1")
    act_y = S("act_y")
    dv_g0, dv_g1 = S("dv_g0"), S("dv_g1")
    dv_c0, dv_c1 = S("dv_c0"), S("dv_c1")
    dv_h0, dv_h1 = S("dv_h0"), S("dv_h1")
    prep = S("prep")
    ls0, ls1 = S("ls0"), S("ls1")
    rs_h0 = [S(f"rs_h0_{p}") for p in range(2)]
    rs_h1 = [S(f"rs_h1_{p}") for p in range(2)]
    dsem = S("dsem")

    N_INIT_DMA = 12
    INIT_V = 16 * N_INIT_DMA

    LS_PER = 16 if one_bcast else 112

    def fills(t):
        # number of exchange rounds into buffer t%2 after step t's exchange
        return t // 2 + 1

    def wtile(sb, k):
        return sb.ap()[:, 512 * k:512 * (k + 1)]

    def htile(sb, k):
        return sb.ap()[:, 64 * k:64 * (k + 1)]

    with nc.Block() as block:

        @block.sync
        def _(eng):
            eng.dma_start(
                w0.ap().rearrange("p (t n) -> p t n", t=12),
                w0_d.ap().rearrange("(t p) n -> p t n", p=128)).then_inc(init, 16)
            eng.dma_start(
                w1.ap().rearrange("p (t n) -> p t n", t=16),
                w1_d.ap().rearrange("(t p) n -> p t n", p=128)).then_inc(init, 16)
            eng.dma_start(
                wfc.ap().rearrange("p (t n) -> p t n", t=8),
                wfc_d.ap().rearrange("(t p) n -> p t n", p=128)).then_inc(init, 16)
            eng.dma_start(b0.ap(), b0_d.ap()).then_inc(init, 16)
            eng.dma_start(b1.ap(), b1_d.ap()).then_inc(init, 16)
            eng.dma_start(bfc.ap(), bfc_d.ap()).then_inc(init, 16)
            eng.dma_start(
                h0T[1].ap().rearrange("p (t n) -> p t n", t=8),
                h0i_d.ap().rearrange("(t p) n -> p t n", p=128)).then_inc(init, 16)
            eng.dma_start(
                h1T[1].ap().rearrange("p (t n) -> p t n", t=8),
                h1i_d.ap().rearrange("(t p) n -> p t n", p=128)).then_inc(init, 16)
            eng.dma_start(cT[0].ap(), c0i_d.ap()).then_inc(init, 16)
            eng.dma_start(cT[1].ap(), c1i_d.ap()).then_inc(init, 16)
            eng.dma_start(
                xT[1].ap().rearrange("p (t n) -> p t n", t=4),
                yi_d.ap().rearrange("(t p) n -> p t n", p=128)).then_inc(init, 16)
            eng.dma_start(ident.ap(), id_d.ap()).then_inc(init, 16)
            for t in range(seq if outdma else 0):
                eng.wait_ge(act_y, t + 1)
                eng.dma_start(
                    out_d.ap()[t], xT[t % 2].ap()).then_inc(dsem, 16)

        @block.tensor
        def _(eng):
            eng.wait_ge(init, INIT_V)
            # prologue: L0 hh-part for t=0 (reads initial h0 in buf 1)
            for k in range(8):
                nc.tensor.matmul(ps_g0.ap(), htile(h0T[1], k), wtile(w0, 4 + k),
                                 start=(k == 0), stop=False)
            for t in range(seq):
                p, q = t % 2, (t + 1) % 2
                # ---- layer 0 gates: close the group with the x-part ----
                if t >= 1:
                    eng.wait_ge(act_y, t)        # x = y(t-1) ready in xT[q]
                for k in range(4):
                    mm = nc.tensor.matmul(ps_g0.ap(), htile(xT[q], k),
                                          wtile(w0, k),
                                          start=False, stop=(k == 3))
                mm.then_inc(pe_g0, 1)
                # early L1-hh matmuls overlap the DVE gate copy
                if t >= 1:
                    eng.wait_ge(dv_g1, t)
                    eng.wait_ge(dv_h1, t)
                    if exchanges and rswaits:
                        eng.wait_ge(rs_h1[q], 14 * fills(t - 1))
                for k in range(3):
                    nc.tensor.matmul(ps_g1.ap(), htile(h1T[q], k),
                                     wtile(w1, 8 + k),
                                     start=(k == 0), stop=False)
                # ---- transpose gates0 to [128, 4*64] ----
                eng.wait_ge(dv_g0, t + 1)        # g0sb written by DVE
                if t >= 1:
                    eng.wait_ge(act_g0, t)       # ps_t0 consumed by ACT
                for j in range(4):
                    mm = nc.tensor.matmul(ps_t0.ap()[:, 64 * j:64 * (j + 1)],
                                          g0sb.ap()[:, 128 * j:128 * (j + 1)],
                                          ident.ap()[:64, :64],
                                          is_transpose=True, start=True,
                                          stop=True)
                mm.then_inc(pe_t0, 1)
                # ---- layer 1 gates: finish hh-part, then fresh-h0 ih-part ----
                for k in range(3, 8):
                    nc.tensor.matmul(ps_g1.ap(), htile(h1T[q], k),
                                     wtile(w1, 8 + k),
                                     start=False, stop=False)
                eng.wait_ge(dv_h0, t + 1)        # own h0(t) slice
                if exchanges and rswaits:
                    eng.wait_ge(rs_h0[p], 14 * fills(t))  # peers' h0(t)
                for k in range(8):
                    mm = nc.tensor.matmul(ps_g1.ap(), htile(h0T[p], k),
                                          wtile(w1, k),
                                          start=False, stop=(k == 7))
                mm.then_inc(pe_g1, 1)
                # ---- transpose gates1 ----
                eng.wait_ge(dv_g1, t + 1)
                if t >= 1:
                    eng.wait_ge(act_g1, t)
                for j in range(4):
                    mm = nc.tensor.matmul(ps_t1.ap()[:, 64 * j:64 * (j + 1)],
                                          g1sb.ap()[:, 128 * j:128 * (j + 1)],
                                          ident.ap()[:64, :64],
                                          is_transpose=True, start=True,
                                          stop=True)
                mm.then_inc(pe_t1, 1)
                # ---- L0 hh-part for step t+1 (fills the h1-exchange window;
                # h0(t) already gathered, ps_g0 drained once dv_g0 hits t+1) ----
                if t + 1 < seq:
                    eng.wait_ge(dv_g0, t + 1)
                    eng.wait_ge(dv_h0, t + 1)
                    if exchanges and rswaits:
                        eng.wait_ge(rs_h0[p], 14 * fills(t))
                    for k in range(8):
                        nc.tensor.matmul(ps_g0.ap(), htile(h0T[p], k),
                                         wtile(w0, 4 + k),
                                         start=(k == 0), stop=False)
                # ---- fc (replicated, weight-stationary): y.T into ps_ty ----
                eng.wait_ge(dv_h1, t + 1)
                if exchanges and rswaits:
                    eng.wait_ge(rs_h1[p], 14 * fills(t))
                if t >= 1:
                    eng.wait_ge(act_y, t)        # ps_ty consumed by ACT(t-1)
                for m in range(4):
                    for k in range(8):
                        mm = nc.tensor.matmul(
                            ps_ty.ap()[:, 64 * m:64 * (m + 1)],
                            wfc.ap()[:, 512 * k + 128 * m:512 * k + 128 * (m + 1)],
                            htile(h1T[p], k),
                            start=(k == 0), stop=(k == 7))
                mm.then_inc(pe_ty, 1)

        @block.scalar
        def _(eng):
            eng.wait_ge(init, INIT_V)
            for t in range(seq):
                p = t % 2
                for l, (ps_t, gsem, thsem, csem, bias) in enumerate(
                        ((ps_t0, act_g0, act_th0, dv_c0, b0),
                         (ps_t1, act_g1, act_th1, dv_c1, b1))):
                    eng.wait_ge((pe_t0, pe_t1)[l], t + 1)
                    i_t, f_t, g_t, o_t, th_t = [x.ap() for x in gl[l][:5]]
                    src = ps_t.ap()
                    a = nc.scalar.activation(i_t, src[:, 0:64], AF.Sigmoid,
                                             bias=bias.ap()[:, 0:1])
                    a = nc.scalar.activation(f_t, src[:, 64:128], AF.Sigmoid,
                                             bias=bias.ap()[:, 1:2])
                    a = nc.scalar.activation(g_t, src[:, 128:192], AF.Tanh,
                                             bias=bias.ap()[:, 2:3])
                    a = nc.scalar.activation(o_t, src[:, 192:256], AF.Sigmoid,
                                             bias=bias.ap()[:, 3:4])
                    a.then_inc(gsem, 1)
                    eng.wait_ge(csem, t + 1)
                    nc.scalar.activation(th_t, cT[l].ap(),
                                         AF.Tanh).then_inc(thsem, 1)
                # fc relu -> xT[p]
                eng.wait_ge(pe_ty, t + 1)
                if t >= 2 and outdma:
                    eng.wait_ge(dsem, 16 * (t - 1))   # out-DMA(t-2) done
                for j in range(4):
                    a = nc.scalar.activation(xT[p].ap()[:, 64 * j:64 * (j + 1)],
                                             ps_ty.ap()[:, 64 * j:64 * (j + 1)],
                                             AF.Relu, bias=bfc.ap()[:, j:j + 1])
                a.then_inc(act_y, 1)

        @block.vector
        def _(eng):
            eng.wait_ge(init, INIT_V)
            dv_off = eng.partition_id() * 64 if one_bcast else None
            for t in range(seq):
                p = t % 2
                # layer 0
                eng.wait_ge(pe_g0, t + 1)
                nc.vector.tensor_copy(g0sb.ap(), ps_g0.ap()).then_inc(dv_g0, 1)
                eng.wait_ge(act_g0, t + 1)
                i_t, f_t, g_t, o_t, th_t, t1, t2 = [x.ap() for x in gl[0]]
                nc.vector.tensor_tensor(t1, f_t, cT[0].ap(), ALU.mult)
                nc.vector.tensor_tensor(t2, i_t, g_t, ALU.mult)
                if t >= 1:
                    eng.wait_ge(act_th0, t)      # tanh(c(t-1)) read done
                nc.vector.tensor_tensor(cT[0].ap(), t1, t2,
                                        ALU.add).then_inc(dv_c0, 1)
                eng.wait_ge(act_th0, t + 1)
                if t >= 2 and exchanges:
                    eng.wait_ge(ls0, LS_PER * (t - 1))  # sends from buf p drained
                h0slot = (h0T[p].ap()[:, bass.ds(dv_off, 64)] if one_bcast
                          else h0T[p].ap()[:, 0:64])
                nc.vector.tensor_tensor(h0slot, o_t, th_t,
                                        ALU.mult).then_inc(dv_h0, 1)
                # layer 1
                eng.wait_ge(pe_g1, t + 1)
                nc.vector.tensor_copy(g1sb.ap(), ps_g1.ap()).then_inc(dv_g1, 1)
                eng.wait_ge(act_g1, t + 1)
                i_t, f_t, g_t, o_t, th_t, t1, t2 = [x.ap() for x in gl[1]]
                nc.vector.tensor_tensor(t1, f_t, cT[1].ap(), ALU.mult)
                nc.vector.tensor_tensor(t2, i_t, g_t, ALU.mult)
                if t >= 1:
                    eng.wait_ge(act_th1, t)
                nc.vector.tensor_tensor(cT[1].ap(), t1, t2,
                                        ALU.add).then_inc(dv_c1, 1)
                eng.wait_ge(act_th1, t + 1)
                if t >= 2 and exchanges:
                    eng.wait_ge(ls1, LS_PER * (t - 1))
                h1slot = (h1T[p].ap()[:, bass.ds(dv_off, 64)] if one_bcast
                          else h1T[p].ap()[:, 0:64])
                nc.vector.tensor_tensor(h1slot, o_t, th_t,
                                        ALU.mult).then_inc(dv_h1, 1)

        @block.gpsimd
        def _(eng):
            eng.wait_ge(init, INIT_V)
            if one_bcast and exchanges:
                gp_off = eng.partition_id() * 64
                rdests = [None] + [(0, d ^ 2) if d >= 4 else (0, d)
                                   for d in range(1, 8)]

                def stage(t):
                    p = t % 2
                    for buf, rsem, lsem in ((h0T[p], rs_h0[p], ls0),
                                            (h1T[p], rs_h1[p], ls1)):
                        slot = buf.ap()[:, bass.ds(gp_off, 64)]
                        eng.remote_dma_broadcast(
                            slot, slot, remote_sem=rsem, local_sem=lsem,
                            rdests=rdests).then_inc(prep, 1)

                stage(0)
                for t in range(seq):
                    eng.wait_ge(prep, 2 * t + 1)
                    eng.wait_ge(dv_h0, t + 1)
                    eng.trigger_dma(count=1)
                    eng.wait_ge(prep, 2 * t + 2)
                    eng.wait_ge(dv_h1, t + 1)
                    eng.trigger_dma(count=1)
                    if t + 1 < seq:
                        stage(t + 1)
            else:
                gp_off = eng.partition_id() * 64 if one_bcast else None
                nprep = 0
                for t in range(seq if exchanges else 0):
                    p = t % 2
                    for buf, hsem, rsem, lsem in ((h0T[p], dv_h0, rs_h0[p], ls0),
                                                  (h1T[p], dv_h1, rs_h1[p], ls1)):
                        eng.wait_ge(hsem, t + 1)
                        for d in range(1, 8):
                            rdests2 = [None] * 8
                            rdests2[d] = (0, d ^ 2) if d >= 4 else (0, d)
                            eng.remote_dma_broadcast(
                                buf.ap()[:, 64 * d:64 * (d + 1)],
                                buf.ap()[:, 0:64],
                                remote_sem=rsem, local_sem=lsem,
                                rdests=rdests2).then_inc(prep, 1)
                        nprep += 7
                        eng.wait_ge(prep, nprep)
                        eng.trigger_dma(count=7)

    nc.compile()
    return nc


def _prep_inputs(core, W_ih0, W_hh0, b_ih0, b_hh0, W_ih1, W_hh1, b_ih1, b_hh1,
                 W_fc, b_fc, h0, c0, rotate=False):
    c = core
    rows = np.concatenate([np.arange(g * H + c * HPC, g * H + (c + 1) * HPC)
                           for g in range(4)])
    if rotate:
        hperm = np.concatenate([np.arange((c ^ j) * HPC, ((c ^ j) + 1) * HPC)
                                for j in range(8)])
    else:
        hperm = np.arange(H)
    import ml_dtypes
    f = np.float32
    bf = ml_dtypes.bfloat16
    w0 = np.concatenate([W_ih0[rows].T, W_hh0[rows].T[hperm]], axis=0)
    w1 = np.concatenate([W_ih1[rows].T[hperm], W_hh1[rows].T[hperm]], axis=0)
    wfc = W_fc.T[hperm]
    return {
        "w0": np.ascontiguousarray(w0).astype(bf),
        "w1": np.ascontiguousarray(w1).astype(bf),
        "wfc": np.ascontiguousarray(wfc).astype(bf),
        "b0": np.ascontiguousarray((b_ih0 + b_hh0)[rows].reshape(4, HPC).T, f),
        "b1": np.ascontiguousarray((b_ih1 + b_hh1)[rows].reshape(4, HPC).T, f),
        "bfc": np.ascontiguousarray(b_fc.reshape(4, HPC).T, f),
        "h0i": np.ascontiguousarray(h0[0].T[hperm]).astype(bf),
        "h1i": np.ascontiguousarray(h0[1].T[hperm]).astype(bf),
        "c0i": np.ascontiguousarray(c0[0][:, c * HPC:(c + 1) * HPC].T, f),
        "c1i": np.ascontiguousarray(c0[1][:, c * HPC:(c + 1) * HPC].T, f),
        "yi": np.full((512, 64), SOS_VALUE, bf),
        "ident": np.eye(128, dtype=f),
    }


def run(seq, in_maps, trace=False, trace_kwargs=None):
    from concourse.bass_utils import run_bass_kernel_spmd

    key = int(seq)
    if key not in _CACHE:
        _CACHE[key] = _build(key)
    nc = _CACHE[key]
    kw = {}
    if trace:
        kw = dict(trace=True, trace_cores=[0], **(trace_kwargs or {}))
    return run_bass_kernel_spmd(nc, in_maps, core_ids=list(range(N_CORES)),
                                **kw)


def kernel(encoder_output=None, h0=None, c0=None, W_ih0=None, W_hh0=None,
           b_ih0=None, b_hh0=None, W_ih1=None, W_hh1=None, b_ih1=None,
           b_hh1=None, W_fc=None, b_fc=None, seq_length=256, _trace=False):
    seq = int(seq_length)
    args = (W_ih0, W_hh0, b_ih0, b_hh0, W_ih1, W_hh1, b_ih1, b_hh1, W_fc, b_fc,
            h0, c0)
    args = tuple(np.asarray(a, np.float32) for a in args)
    in_maps = [_prep_inputs(c, *args) for c in range(N_CORES)]
    res = run(seq, in_maps, trace=_trace)
    out = np.asarray(res.results[0]["out"]).astype(np.float32)
    y = out.reshape(seq, 128, 4, 64).transpose(3, 0, 2, 1).reshape(B, seq, OUT)
    if _trace:
        kernel._last_results = res
    return np.ascontiguousarray(y)



# revision 17
# speedup vs baseline: 2.0931x; 2.0931x over previous
"""Trainium2 Bass kernel for a 2-layer LSTM decoder (nn_Decoder).

Strategy: tensor-parallel over the hidden dimension across 8 NeuronCores.
Each core owns a 128-wide slice of H=1024 for both LSTM layers (its 512 of
the 4096 gate rows), and replicates the final fc layer so the autoregressive
input y needs no exchange.  The only cross-core traffic is an allgather of
each layer's hidden-state slice (128x64 bf16 = 16KB) per step, done with
direct SBUF->SBUF remote DMA (no HBM bounce, no ncfw collective floor).

v2 layout: gates are computed directly in feature-major orientation
out[gate_rows(128), batch(64)] with the weight tile as the stationary
operand and the (transposed) activations as the 64-row moving operand.
This halves the PE row count vs the moving-weight orientation (the PE
charges free-dim rows regardless of partition fill), and the gate chunks
land in PSUM exactly in the [hid, batch] layout the cell update wants --
no PE transpose, no PSUM->SBUF gate copy.

Gate rows are host-reordered to (f, i, o, g) so one fused Sigmoid over
[128, 192] covers f,i,o and one Tanh covers g.  Biases are folded into
the PSUM accumulation with K=1 matmuls (lhsT = bias row, rhs = ones row),
which is what makes the fused activations legal.  PSUM banks are split
(f,i,o | g per layer, fc halves) so an activation never reads a bank
that later matmuls of the same step still accumulate into.

The fc runs weight-stationary into two PSUM banks so relu of the first
256 output rows overlaps the matmuls of the last 256 (and layer-0's
x-part matmuls start after the first relu half).

Each exchange is ONE remote_dma_broadcast to the 7 peers (XOR-relative
routing; empirically the D2D lanes (delta bit 2 set) deliver to delta^2 so
those rdests are pre-compensated).  The destination slot is sender-pid *
64 columns via a register access pattern, so every core runs an identical
program.  Descriptor generation is pre-staged one step ahead on the Q7;
only trigger_dma sits on the critical path.

Flow control: h buffers are double-buffered by step parity; the LSTM's own
data-dependency chain guarantees a sender cannot run 2 steps ahead of any
receiver, so no credit messages are needed.
"""

import sys

sys.path.insert(0, "/opt/trn_rl_repo")

import numpy as np

B = 64
H = 1024
OUT = 512
N_CORES = 8
HPC = H // N_CORES  # 128 hidden units per core
SOS_VALUE = -2.0

_CACHE = {}


def _build(seq):
    from concourse import bacc, bass, mybir

    dt = mybir.dt
    f32 = dt.float32
    bf16 = dt.bfloat16
    AF = mybir.ActivationFunctionType
    ALU = mybir.AluOpType

    nc = bacc.Bacc("TRN2", target_bir_lowering=False, debug=False,
                   num_devices=N_CORES)

    # ---- DRAM I/O (per-core shards prepared on host) ----
    w0x_d = nc.dram_tensor("w0x", [4 * 128, 512], bf16, kind="ExternalInput")
    w0h_d = nc.dram_tensor("w0h", [8 * 128, 512], bf16, kind="ExternalInput")
    w1_d = nc.dram_tensor("w1", [16 * 128, 512], bf16, kind="ExternalInput")
    wfc_d = nc.dram_tensor("wfc", [8 * 128, 512], bf16, kind="ExternalInput")
    b0_d = nc.dram_tensor("b0", [1, 512], bf16, kind="ExternalInput")
    b1_d = nc.dram_tensor("b1", [1, 512], bf16, kind="ExternalInput")
    bfc_d = nc.dram_tensor("bfc", [1, 512], bf16, kind="ExternalInput")
    ones_d = nc.dram_tensor("ones", [1, 64], bf16, kind="ExternalInput")
    h0i_d = nc.dram_tensor("h0i", [1024, 64], bf16, kind="ExternalInput")
    h1i_d = nc.dram_tensor("h1i", [1024, 64], bf16, kind="ExternalInput")
    c0i_d = nc.dram_tensor("c0i", [128, 64], f32, kind="ExternalInput")
    c1i_d = nc.dram_tensor("c1i", [128, 64], f32, kind="ExternalInput")
    yi_d = nc.dram_tensor("yi", [512, 64], bf16, kind="ExternalInput")
    out_d = nc.dram_tensor("out", [seq, 128, 256], bf16, kind="ExternalOutput")

    # ---- SBUF ----
    w0x = nc.alloc_sbuf_tensor("w0xs", [128, 4 * 512], bf16)
    w0h = nc.alloc_sbuf_tensor("w0hs", [128, 8 * 512], bf16)
    w1 = nc.alloc_sbuf_tensor("w1s", [128, 16 * 512], bf16)
    wfc = nc.alloc_sbuf_tensor("wfcs", [128, 8 * 512], bf16)
    b0 = nc.alloc_sbuf_tensor("b0s", [1, 512], bf16)
    b1 = nc.alloc_sbuf_tensor("b1s", [1, 512], bf16)
    bfc = nc.alloc_sbuf_tensor("bfcs", [1, 512], bf16)
    ones = nc.alloc_sbuf_tensor("oness", [1, 64], bf16)
    h0T = [nc.alloc_sbuf_tensor(f"h0T{p}", [128, 512], bf16) for p in range(2)]
    h1T = [nc.alloc_sbuf_tensor(f"h1T{p}", [128, 512], bf16) for p in range(2)]
    xT = [nc.alloc_sbuf_tensor(f"xT{p}", [128, 256], bf16) for p in range(2)]
    # per-layer cell-state scratch: sbg = sigmoid(f,i,o); cg = [c | tanh(g)]
    sbg = [nc.alloc_sbuf_tensor(f"sbg{l}", [128, 192], f32) for l in range(2)]
    cg = [nc.alloc_sbuf_tensor(f"cg{l}", [128, 128], f32) for l in range(2)]
    th = [nc.alloc_sbuf_tensor(f"th{l}", [128, 64], f32) for l in range(2)]
    tmp = [[nc.alloc_sbuf_tensor(f"tmp{l}_{j}", [128, 64], f32)
            for j in range(2)] for l in range(2)]

    # ---- PSUM (single-buffered; bank-split so ACT never reads a bank
    # other matmuls of the same step still write) ----
    ps_g0a = nc.alloc_psum_tensor("ps_g0a", [128, 192], f32)  # f,i,o
    ps_g0b = nc.alloc_psum_tensor("ps_g0b", [128, 64], f32)   # g
    ps_g1a = nc.alloc_psum_tensor("ps_g1a", [128, 192], f32)
    ps_g1b = nc.alloc_psum_tensor("ps_g1b", [128, 64], f32)
    ps_ya = nc.alloc_psum_tensor("ps_ya", [128, 128], f32)    # y rows 0:256
    ps_yb = nc.alloc_psum_tensor("ps_yb", [128, 128], f32)    # y rows 256:512

    # ---- semaphores ----
    S = lambda n: nc.alloc_semaphore(n)
    init = S("init")
    pe_g0a, pe_g0b = S("pe_g0a"), S("pe_g0b")
    pe_g1a, pe_g1b = S("pe_g1a"), S("pe_g1b")
    pe_ya, pe_yb = S("pe_ya"), S("pe_yb")
    act_s0, act_tg0, act_tc0 = S("act_s0"), S("act_tg0"), S("act_tc0")
    act_s1, act_tg1, act_tc1 = S("act_s1"), S("act_tg1"), S("act_tc1")
    act_ya, act_yb = S("act_ya"), S("act_yb")
    dv_c0, dv_c1 = S("dv_c0"), S("dv_c1")
    dv_h0, dv_h1 = S("dv_h0"), S("dv_h1")
    prep = S("prep")
    ls0, ls1 = S("ls0"), S("ls1")
    rs_h0 = [S(f"rs_h0_{p}") for p in range(2)]
    rs_h1 = [S(f"rs_h1_{p}") for p in range(2)]
    dsem = S("dsem")

    N_INIT_DMA = 13
    INIT_V = 16 * N_INIT_DMA

    def fills(t):
        # number of exchange rounds into buffer t%2 after step t's exchange
        return t // 2 + 1

    # weight tile (k-chunk k, gate/out chunk g) as the stationary lhsT
    def wt(sb, k, g):
        return sb.ap()[:, 512 * k + 128 * g:512 * k + 128 * (g + 1)]

    def wtx(k, g):
        return wt(w0x, k, g)

    def htile(sb, k):
        return sb.ap()[:, 64 * k:64 * (k + 1)]

    # psum region for gate q (0=f,1=i,2=o in the a-bank; 3=g in the b-bank)
    def greg(psa, psb, g):
        return psb.ap() if g == 3 else psa.ap()[:, 64 * g:64 * (g + 1)]

    with nc.Block() as block:

        @block.sync
        def _(eng):
            eng.dma_start(
                w0x.ap().rearrange("p (t n) -> p t n", t=4),
                w0x_d.ap().rearrange("(t p) n -> p t n", p=128)).then_inc(init, 16)
            eng.dma_start(
                w0h.ap().rearrange("p (t n) -> p t n", t=8),
                w0h_d.ap().rearrange("(t p) n -> p t n", p=128)).then_inc(init, 16)
            eng.dma_start(
                w1.ap().rearrange("p (t n) -> p t n", t=16),
                w1_d.ap().rearrange("(t p) n -> p t n", p=128)).then_inc(init, 16)
            eng.dma_start(
                wfc.ap().rearrange("p (t n) -> p t n", t=8),
                wfc_d.ap().rearrange("(t p) n -> p t n", p=128)).then_inc(init, 16)
            eng.dma_start(b0.ap(), b0_d.ap()).then_inc(init, 16)
            eng.dma_start(b1.ap(), b1_d.ap()).then_inc(init, 16)
            eng.dma_start(bfc.ap(), bfc_d.ap()).then_inc(init, 16)
            eng.dma_start(ones.ap(), ones_d.ap()).then_inc(init, 16)
            eng.dma_start(
                h0T[1].ap().rearrange("p (t n) -> p t n", t=8),
                h0i_d.ap().rearrange("(t p) n -> p t n", p=128)).then_inc(init, 16)
            eng.dma_start(
                h1T[1].ap().rearrange("p (t n) -> p t n", t=8),
                h1i_d.ap().rearrange("(t p) n -> p t n", p=128)).then_inc(init, 16)
            eng.dma_start(cg[0].ap()[:, 0:64], c0i_d.ap()).then_inc(init, 16)
            eng.dma_start(cg[1].ap()[:, 0:64], c1i_d.ap()).then_inc(init, 16)
            eng.dma_start(
                xT[1].ap().rearrange("p (t n) -> p t n", t=4),
                yi_d.ap().rearrange("(t p) n -> p t n", p=128)).then_inc(init, 16)
            for t in range(seq):
                eng.wait_ge(act_yb, t + 1)
                eng.dma_start(
                    out_d.ap()[t], xT[t % 2].ap()).then_inc(dsem, 16)

        @block.tensor
        def _(eng):
            eng.wait_ge(init, INIT_V)

            def prefill_l0(hbuf):
                # bias + hh-part of next step's layer-0 gates; one
                # accumulation group per bank (start on the first mm)
                for g in range(4):
                    nc.tensor.matmul(greg(ps_g0a, ps_g0b, g),
                                     b0.ap()[:, 128 * g:128 * (g + 1)],
                                     ones.ap(), start=(g in (0, 3)),
                                     stop=False)
                for g in range(4):
                    for k in range(8):
                        nc.tensor.matmul(greg(ps_g0a, ps_g0b, g),
                                         wt(w0h, k, g), htile(hbuf, k),
                                         start=False, stop=False)

            prefill_l0(h0T[1])
            for t in range(seq):
                p, q = t % 2, (t + 1) % 2
                # ---- layer 0 gates: close with the x-part ----
                if t >= 1:
                    eng.wait_ge(act_ya, t)       # y(t-1) rows 0:256 in xT[q]
                for g in range(3):
                    for k in range(2):
                        nc.tensor.matmul(greg(ps_g0a, ps_g0b, g),
                                         wtx(k, g), htile(xT[q], k),
                                         start=False, stop=False)
                if t >= 1:
                    eng.wait_ge(act_yb, t)       # y(t-1) rows 256:512
                for g in range(3):
                    for k in range(2, 4):
                        mm = nc.tensor.matmul(greg(ps_g0a, ps_g0b, g),
                                              wtx(k, g), htile(xT[q], k),
                                              start=False,
                                              stop=(g == 2 and k == 3))
                mm.then_inc(pe_g0a, 1)
                for k in range(4):
                    mm = nc.tensor.matmul(greg(ps_g0a, ps_g0b, 3),
                                          wtx(k, 3), htile(xT[q], k),
                                          start=False, stop=(k == 3))
                mm.then_inc(pe_g0b, 1)
                # ---- layer 1: bias + hh-part (overlaps L0's cell chain) ----
                if t >= 2:
                    eng.wait_ge(act_tg1, t - 1)  # ACT(t-2) done with ps_g1
                if t >= 1:
                    eng.wait_ge(dv_h1, t)
                    eng.wait_ge(rs_h1[q], 14 * fills(t - 1))
                for g in range(4):
                    nc.tensor.matmul(greg(ps_g1a, ps_g1b, g),
                                     b1.ap()[:, 128 * g:128 * (g + 1)],
                                     ones.ap(), start=(g in (0, 3)),
                                     stop=False)
                for g in range(4):
                    for k in range(8):
                        nc.tensor.matmul(greg(ps_g1a, ps_g1b, g),
                                         wt(w1, 8 + k, g), htile(h1T[q], k),
                                         start=False, stop=False)
                # ---- layer 1 ih-part: needs gathered h0(t) ----
                eng.wait_ge(dv_h0, t + 1)
                eng.wait_ge(rs_h0[p], 14 * fills(t))
                for g in range(3):
                    for k in range(8):
                        mm = nc.tensor.matmul(greg(ps_g1a, ps_g1b, g),
                                              wt(w1, k, g), htile(h0T[p], k),
                                              start=False,
                                              stop=(g == 2 and k == 7))
                mm.then_inc(pe_g1a, 1)
                for k in range(8):
                    mm = nc.tensor.matmul(greg(ps_g1a, ps_g1b, 3),
                                          wt(w1, k, 3), htile(h0T[p], k),
                                          start=False, stop=(k == 7))
                mm.then_inc(pe_g1b, 1)
                # ---- L0 prefill for t+1 (fills the h1-exchange window) ----
                if t + 1 < seq:
                    eng.wait_ge(act_tg0, t + 1)  # ACT(t) done with ps_g0
                    prefill_l0(h0T[p])
                # ---- fc (replicated, weight-stationary) ----
                eng.wait_ge(dv_h1, t + 1)
                eng.wait_ge(rs_h1[p], 14 * fills(t))
                if t >= 1:
                    eng.wait_ge(act_ya, t)       # relu(t-1) done with ps_ya
                for m in range(2):
                    nc.tensor.matmul(ps_ya.ap()[:, 64 * m:64 * (m + 1)],
                                     bfc.ap()[:, 128 * m:128 * (m + 1)],
                                     ones.ap(), start=(m == 0), stop=False)
                for m in range(2):
                    for k in range(8):
                        mm = nc.tensor.matmul(
                            ps_ya.ap()[:, 64 * m:64 * (m + 1)],
                            wt(wfc, k, m), htile(h1T[p], k),
                            start=False, stop=(m == 1 and k == 7))
                mm.then_inc(pe_ya, 1)
                if t >= 1:
                    eng.wait_ge(act_yb, t)
                for m in range(2, 4):
                    nc.tensor.matmul(ps_yb.ap()[:, 64 * (m - 2):64 * (m - 1)],
                                     bfc.ap()[:, 128 * m:128 * (m + 1)],
                                     ones.ap(), start=(m == 2), stop=False)
                for m in range(2, 4):
                    for k in range(8):
                        mm = nc.tensor.matmul(
                            ps_yb.ap()[:, 64 * (m - 2):64 * (m - 1)],
                            wt(wfc, k, m), htile(h1T[p], k),
                            start=False, stop=(m == 3 and k == 7))
                mm.then_inc(pe_yb, 1)

        @block.scalar
        def _(eng):
            eng.wait_ge(init, INIT_V)
            for t in range(seq):
                p = t % 2
                for l, (psa, psb, sem_a, sem_b, s_s, s_tg, s_tc, dcs) in (
                        (0, (ps_g0a, ps_g0b, pe_g0a, pe_g0b,
                             act_s0, act_tg0, act_tc0, dv_c0)),
                        (1, (ps_g1a, ps_g1b, pe_g1a, pe_g1b,
                             act_s1, act_tg1, act_tc1, dv_c1))):
                    eng.wait_ge(sem_a, t + 1)
                    nc.scalar.activation(sbg[l].ap(), psa.ap(),
                                         AF.Sigmoid).then_inc(s_s, 1)
                    eng.wait_ge(sem_b, t + 1)
                    nc.scalar.activation(cg[l].ap()[:, 64:128], psb.ap(),
                                         AF.Tanh).then_inc(s_tg, 1)
                    eng.wait_ge(dcs, t + 1)
                    nc.scalar.activation(th[l].ap(), cg[l].ap()[:, 0:64],
                                         AF.Tanh).then_inc(s_tc, 1)
                # fc relu halves -> xT[p]
                eng.wait_ge(pe_ya, t + 1)
                if t >= 2:
                    eng.wait_ge(dsem, 16 * (t - 1))   # out-DMA(t-2) done
                nc.scalar.activation(xT[p].ap()[:, 0:128], ps_ya.ap(),
                                     AF.Relu).then_inc(act_ya, 1)
                eng.wait_ge(pe_yb, t + 1)
                nc.scalar.activation(xT[p].ap()[:, 128:256], ps_yb.ap(),
                                     AF.Relu).then_inc(act_yb, 1)

        @block.vector
        def _(eng):
            eng.wait_ge(init, INIT_V)
            dv_off = eng.partition_id() * 64
            for t in range(seq):
                p = t % 2
                for l, (s_s, s_tg, s_tc, dcs, dhs, hbuf, lsem) in (
                        (0, (act_s0, act_tg0, act_tc0, dv_c0, dv_h0,
                             h0T[p], ls0)),
                        (1, (act_s1, act_tg1, act_tc1, dv_c1, dv_h1,
                             h1T[p], ls1))):
                    c_ap = cg[l].ap()[:, 0:64]
                    eng.wait_ge(s_s, t + 1)
                    nc.vector.tensor_tensor(tmp[l][0].ap(),
                                            sbg[l].ap()[:, 0:64], c_ap,
                                            ALU.mult)          # f * c
                    eng.wait_ge(s_tg, t + 1)
                    nc.vector.tensor_tensor(tmp[l][1].ap(),
                                            sbg[l].ap()[:, 64:128],
                                            cg[l].ap()[:, 64:128],
                                            ALU.mult)          # i * tanh(g)
                    if t >= 1:
                        eng.wait_ge(s_tc, t)   # tanh(c(t-1)) read done
                    nc.vector.tensor_tensor(c_ap, tmp[l][0].ap(),
                                            tmp[l][1].ap(),
                                            ALU.add).then_inc(dcs, 1)
                    eng.wait_ge(s_tc, t + 1)
                    if t >= 2:
                        eng.wait_ge(lsem, 16 * (t - 1))  # bcast(t-2) drained
                    nc.vector.tensor_tensor(hbuf.ap()[:, bass.ds(dv_off, 64)],
                                            sbg[l].ap()[:, 128:192],
                                            th[l].ap(),
                                            ALU.mult).then_inc(dhs, 1)

        @block.gpsimd
        def _(eng):
            eng.wait_ge(init, INIT_V)
            gp_off = eng.partition_id() * 64
            rdests = [None] + [(0, d ^ 2) if d >= 4 else (0, d)
                               for d in range(1, 8)]

            def stage(t):
                p = t % 2
                for buf, rsem, lsem in ((h0T[p], rs_h0[p], ls0),
                                        (h1T[p], rs_h1[p], ls1)):
                    slot = buf.ap()[:, bass.ds(gp_off, 64)]
                    eng.remote_dma_broadcast(
                        slot, slot, remote_sem=rsem, local_sem=lsem,
                        rdests=rdests).then_inc(prep, 1)

            stage(0)
            for t in range(seq):
                eng.wait_ge(prep, 2 * t + 1)
                eng.wait_ge(dv_h0, t + 1)
                eng.trigger_dma(count=1)
                eng.wait_ge(prep, 2 * t + 2)
                eng.wait_ge(dv_h1, t + 1)
                eng.trigger_dma(count=1)
                if t + 1 < seq:
                    stage(t + 1)

    nc.compile()
    return nc


def _prep_inputs(core, W_ih0, W_hh0, b_ih0, b_hh0, W_ih1, W_hh1, b_ih1, b_hh1,
                 W_fc, b_fc, h0, c0):
    c = core
    # gate-chunk order (f, i, o, g); PyTorch row order is (i, f, g, o)
    rows = np.concatenate([np.arange(g * H + c * HPC, g * H + (c + 1) * HPC)
                           for g in (1, 0, 3, 2)])
    import ml_dtypes
    f = np.float32
    bf = ml_dtypes.bfloat16
    w1 = np.concatenate([W_ih1[rows].T, W_hh1[rows].T], axis=0)
    return {
        "w0x": np.ascontiguousarray(W_ih0[rows].T).astype(bf),
        "w0h": np.ascontiguousarray(W_hh0[rows].T).astype(bf),
        "w1": np.ascontiguousarray(w1).astype(bf),
        "wfc": np.ascontiguousarray(W_fc.T).astype(bf),
        "b0": np.ascontiguousarray((b_ih0 + b_hh0)[rows][None, :]).astype(bf),
        "b1": np.ascontiguousarray((b_ih1 + b_hh1)[rows][None, :]).astype(bf),
        "bfc": np.ascontiguousarray(b_fc[None, :]).astype(bf),
        "ones": np.ones((1, 64), bf),
        "h0i": np.ascontiguousarray(h0[0].T).astype(bf),
        "h1i": np.ascontiguousarray(h0[1].T).astype(bf),
        "c0i": np.ascontiguousarray(c0[0][:, c * HPC:(c + 1) * HPC].T, f),
        "c1i": np.ascontiguousarray(c0[1][:, c * HPC:(c + 1) * HPC].T, f),
        "yi": np.full((512, 64), SOS_VALUE, bf),
    }


def run(seq, in_maps, trace=False, trace_kwargs=None):
    from concourse.bass_utils import run_bass_kernel_spmd

    key = int(seq)
    if key not in _CACHE:
        _CACHE[key] = _build(key)
    nc = _CACHE[key]
    kw = {}
    if trace:
        kw = dict(trace=True, trace_cores=[0], **(trace_kwargs or {}))
    return run_bass_kernel_spmd(nc, in_maps, core_ids=list(range(N_CORES)),
                                **kw)


def kernel(encoder_output=None, h0=None, c0=None, W_ih0=None, W_hh0=None,
           b_ih0=None, b_hh0=None, W_ih1=None, W_hh1=None, b_ih1=None,
           b_hh1=None, W_fc=None, b_fc=None, seq_length=256, _trace=False):
    seq = int(seq_length)
    args = (W_ih0, W_hh0, b_ih0, b_hh0, W_ih1, W_hh1, b_ih1, b_hh1, W_fc, b_fc,
            h0, c0)
    args = tuple(np.asarray(a, np.float32) for a in args)
    in_maps = [_prep_inputs(c, *args) for c in range(N_CORES)]
    res = run(seq, in_maps, trace=_trace)
    out = np.asarray(res.results[0]["out"]).astype(np.float32)
    y = out.reshape(seq, 128, 4, 64).transpose(3, 0, 2, 1).reshape(B, seq, OUT)
    if _trace:
        kernel._last_results = res
    return np.ascontiguousarray(y)
